# revision 54
# baseline (speedup 1.0000x reference)
"""MoE layer (dense-gated top-2 of 8 experts) on 8 trn2 NeuronCores.

Strategy: expert-parallel SPMD with host-side routing. The host computes the
gate (LN -> logits -> softmax -> top-2 -> renormalized weights) exactly in
fp32, then for each expert pre-gathers its selected token rows and lays them
out directly in the fp8 DoubleRow rhs format the PE consumes (xeT8), plus an
fp16 row-major copy (xg) for the residual. Core e runs a pure dense FFN over
its C compact slots (C = max expert count rounded up to 128, derived from the
actual routing at build time) and returns z = h@W2/S2 + x in f16; the output
LayerNorm, gate weighting, ln affine and scatter-add combine run on the host
(which already owns the unshard step).

Device pipeline, one-chunk skew so the PE never waits on the Act engine:
  mm1(s):   ps1 = (32*W1)^T @ x   (fp8 DoubleRow, fp32 accum)
            hT  = gelu(ps1/32)    (Act, fp8 out, DoubleRow-interleaved)
  mm2(s-1): ps2 = hT^T @ (32*W2)  (fp8 DoubleRow)
            z   = ps2/32 + xg     (DVE, f16 out) -> DMA out.

W1 is loaded as 16 quarter-tiles ordered so the first matmul can start after
~1/4 of the weight bytes have landed; xeT8 is staged per chunk.
"""

import numpy as np
import ml_dtypes

import concourse.bass as bass
import concourse.mybir as mybir
import concourse.tile as tile
from concourse.vector_clock import ScopedClock
from concourse.bass_utils import run_bass_kernel_spmd

f32 = mybir.dt.float32
bf16 = mybir.dt.bfloat16
f16 = mybir.dt.float16
f8 = mybir.dt.float8e4
i32 = mybir.dt.int32
AF = mybir.ActivationFunctionType
OP = mybir.AluOpType
AX = mybir.AxisListType
PM = mybir.MatmulPerfMode

# ---------------------------------------------------------------------------
# Walrus workaround: this toolchain supports at most ONE sync wait per
# instruction; split excess waits onto same-engine nops inserted just before.
# ---------------------------------------------------------------------------
_ctr = [0]


def _mknop(engine, waits):
    _ctr[0] += 1
    n = mybir.InstNoOp(name=f"waitsplit-{_ctr[0]}", ins=[], outs=[])
    n.engine = engine
    n.sync_info = mybir.SyncInfo(on_wait=list(waits), on_update=[])
    return n


def split_sync_waits(nc, maxw=1):
    for f in nc.m.functions:
        for blk in f.blocks:
            insts = list(blk.instructions)
            if not any(
                (i.sync_info is not None and i.sync_info.on_wait
                 and len(i.sync_info.on_wait) > maxw)
                for i in insts
            ):
                continue
            out = []
            for ins in insts:
                si = ins.sync_info
                if si is not None and si.on_wait and len(si.on_wait) > maxw:
                    waits = list(si.on_wait)
                    for i in range(0, len(waits) - maxw, maxw):
                        out.append(_mknop(ins.engine, waits[i:i + maxw]))
                    ins.sync_info = mybir.SyncInfo(
                        on_wait=waits[len(waits) - maxw:],
                        on_update=list(si.on_update or []))
                out.append(ins)
            blk.instructions = out


def _drain_and_barrier(self, tick_clock, wait_clock):
    nop0 = self.nc.sync.nop(nofuse=True)
    wait_clock.add_sem_waits(nop0.ins, ScopedClock({None: tick_clock.global_clock}))
    si = nop0.ins.sync_info
    if si is not None and si.on_wait and len(si.on_wait) > 1:
        waits = list(si.on_wait)
        nop0.ins.sync_info = mybir.SyncInfo(
            on_wait=waits[:1], on_update=list(si.on_update or []))
        for w in waits[1:]:
            n = self.nc.sync.nop(nofuse=True)
            n.ins.sync_info = mybir.SyncInfo(on_wait=[w], on_update=[])
    self.nc.sync.drain()
    self.nc.all_engine_barrier()
    assert self.sems is not None
    popped = self.nc._tile_sem_poison_stack.pop()
    assert popped is self._sem_poison
    self.nc.clear_and_free_semaphores(list(self.sems.allocated().values()))
    self.nc.all_engine_barrier()


tile.TileContext._drain_and_barrier = _drain_and_barrier

# ---------------------------------------------------------------------------
# Problem constants
# ---------------------------------------------------------------------------
B, S, H, F, E, K = 4, 2048, 1024, 4096, 8, 2
T_FULL = B * S            # 8192 tokens
LN_EPS = 1e-5
S1 = 32.0                 # fp8 pre-scale for W1
S2 = 32.0                 # fp8 pre-scale for W2


def _chunks_of(C):
    out = []
    base = 0
    while base < C:
        n = min(512, C - base)
        out.append((base, n))
        base += n
    return out


def build_nc(C, use_b1=False, split=True):
    _ctr[0] = 0              # deterministic module content -> NEFF cache hits
    chunks = _chunks_of(C)
    NCH = len(chunks)

    nc = bass.Bass(trn_type="TRN2")

    # ---- I/O ----
    # xeT8[p, j2*2C + q*C + t] = fp8(x[ids[t], j2*256 + q*128 + p])
    xeT8 = nc.dram_tensor("xeT8", (128, 8 * C), f8, kind="ExternalInput")
    # gathered rows (residual), fp16
    xg = nc.dram_tensor("xg", (C, H), f16, kind="ExternalInput")
    # W1/W2 pre-scaled fp8 in DoubleRow pair-interleaved layout:
    # w1p[j2*128+p, q*F+f] = fp8(W1[j2*256+q*128+p, f]*S1)
    w1p = nc.dram_tensor("w1p", (4 * 128, 2 * F), f8, kind="ExternalInput")
    # w2p[k*128+p, q*H+h] = fp8(W2[k*256+q*128+p, h]*S2)
    w2p = nc.dram_tensor("w2p", (16 * 128, 2 * H), f8, kind="ExternalInput")
    if use_b1:
        b1t = nc.dram_tensor("b1t", (128, F // 128), f32, kind="ExternalInput")

    Yc = nc.dram_tensor("Yc", (C, H), f16, kind="ExternalOutput")

    with tile.TileContext(nc) as tc:
        with tc.tile_pool(name="persist", bufs=1) as pp, \
             tc.tile_pool(name="pxt", bufs=3) as pxt, \
             tc.tile_pool(name="pxg", bufs=3) as pxg, \
             tc.tile_pool(name="phT", bufs=2) as phT, \
             tc.tile_pool(name="prs", bufs=3) as prs, \
             tc.tile_pool(name="ps1", bufs=2, space="PSUM") as psc1:

            xeT8_0 = xeT8[:]
            xg0 = xg[:]
            yc0 = Yc[:]
            w1p0 = w1p[:]
            state_xt = {}
            state_xg = {}
            hts = {}

            # warm the Act gelu table at t=0 so the first real gelu doesn't
            # pay the 1283ns table load mid-pipeline
            warm = pp.tile([1, 8], f32, tag="warm")
            nc.vector.memset(warm[:], 0.0)
            warm2 = pp.tile([1, 8], f32, tag="warm2")
            nc.scalar.activation(out=warm2[:], in_=warm[:], func=AF.Gelu)

            def stage_xt(s, engine):
                # per-chunk mm1 rhs: tile cols j2*2n + q*n + t
                base, n = chunks[s]
                xt_t = pxt.tile([128, 8 * 512], f8, tag="xt", name=f"xt_{s}")
                engine.dma_start(
                    out=bass.AP(xt_t.tensor, xt_t[:].offset,
                                [[8 * 512, 128], [2 * n, 4], [n, 2], [1, n]]),
                    in_=bass.AP(xeT8_0.tensor, xeT8_0.offset + base,
                                [[8 * C, 128], [2 * C, 4], [C, 2], [1, n]]))
                state_xt[s] = xt_t

            def stage_xg(s, engine):
                base, n = chunks[s]
                tt = n // 128
                xg_t = pxg.tile([128, 4 * H], f16, tag="xg", name=f"xg_{s}")
                engine.dma_start(
                    out=xg_t[:, 0:tt * H].rearrange("p (t h) -> p t h", h=H),
                    in_=bass.AP(xg0.tensor, xg0.offset + base * H,
                                [[H, 128], [128 * H, tt], [1, H]]))
                state_xg[s] = xg_t

            # ---- DMA issue order shapes the ramp ----
            # Pool queue: w1 eighths in i-consumption order, then xt1/w2/xg.
            w1q = [[None] * 8 for _ in range(4)]   # [j2][o]
            stage_xt(0, nc.sync)
            for o in range(8):
                for j2 in range(4):
                    t = pp.tile([128, 1024], f8, tag=f"w1_{j2}_{o}",
                                name=f"w1q_{j2}_{o}")
                    w1q[j2][o] = t
                    nc.gpsimd.dma_start(
                        out=t[:],
                        in_=bass.AP(w1p0.tensor,
                                    w1p0.offset + (j2 * 128) * (2 * F) + o * 512,
                                    [[2 * F, 128], [F, 2], [1, 512]]))
            if NCH > 1:
                stage_xt(1, nc.gpsimd)
            w2_sb = []
            for k in range(16):
                t = pp.tile([128, 2 * H], f8, tag=f"w2_{k}", name=f"w2sb_{k}")
                w2_sb.append(t)
                nc.gpsimd.dma_start(out=t[:], in_=w2p[k * 128:(k + 1) * 128, :])
            stage_xg(0, nc.gpsimd)
            if NCH > 1:
                stage_xg(1, nc.gpsimd)
            if use_b1:
                b1t_sb = pp.tile([128, F // 128], f32, tag="b1t")
                nc.gpsimd.dma_start(out=b1t_sb[:], in_=b1t[:])

            def stage_mm1(s, ring):
                base, n = chunks[s]
                xt_t = state_xt[s]
                hTb = phT.tile([128, 32 * 512], f8, tag="hT", name=f"hT_{s}")
                hts[s] = hTb
                for i2 in range(16):
                    # 2-bank PSUM tile: two i-slices share one gelu, halving
                    # the Act per-instruction init overhead
                    pool, ptag = ring[i2 % len(ring)]
                    ps1 = pool.tile([128, 1024], f32, tag=ptag,
                                    name=f"ps1_{s}_{i2}")
                    for j2 in range(4):
                        rhs = xt_t[:, j2 * 2 * n:(j2 + 1) * 2 * n].rearrange(
                            "p (two t) -> p two t", two=2)
                        for sub in range(2):
                            i = 2 * i2 + sub
                            lhsT = w1q[j2][i // 4][:].rearrange(
                                "p (two f) -> p two f", two=2)[
                                :, :, (i % 4) * 128:(i % 4 + 1) * 128]
                            nc.tensor.matmul(
                                out=ps1[:, sub * 512:sub * 512 + n],
                                lhsT=lhsT, rhs=rhs,
                                start=(j2 == 0), stop=(j2 == 3),
                                perf_mode=PM.DoubleRow)
                    if use_b1:
                        for sub in range(2):
                            i = 2 * i2 + sub
                            nc.scalar.activation(
                                out=hTb[:, i * 512:i * 512 + n],
                                in_=ps1[:, sub * 512:sub * 512 + n],
                                func=AF.Gelu,
                                bias=b1t_sb[:, i:i + 1], scale=1.0 / S1)
                    else:
                        nc.scalar.activation(
                            out=hTb[:, (2 * i2) * 512:(2 * i2 + 2) * 512].rearrange(
                                "p (two t) -> p two t", two=2)[:, :, 0:n],
                            in_=ps1[:].rearrange(
                                "p (two t) -> p two t", two=2)[:, :, 0:n],
                            func=AF.Gelu, scale=1.0 / S1)

            def stage_mm2_z(s, psc2):
                base, n = chunks[s]
                tt = n // 128
                hTb = hts.pop(s)
                xg_t = state_xg.pop(s)
                tgroups = [list(range(g, min(g + 2, tt)))
                           for g in range(0, tt, 2)]
                def mm2_chain(ps2t, t, c0, cw, name_sfx):
                    # one k-accumulation chain covering out columns [c0,c0+cw)
                    for k in range(16):
                        lhsT = hTb[:, 2 * k * 512:
                                   2 * k * 512 + 1024].rearrange(
                            "p (two tx) -> p two tx", two=2)[
                            :, :, t * 128:(t + 1) * 128]
                        rhs = w2_sb[k][:].rearrange(
                            "p (two h) -> p two h", two=2)[:, :, c0:c0 + cw]
                        nc.tensor.matmul(
                            out=ps2t[:, c0 - (c0 // 512) * 512:
                                     c0 - (c0 // 512) * 512 + cw],
                            lhsT=lhsT, rhs=rhs,
                            start=(k == 0), stop=(k == 15),
                            perf_mode=PM.DoubleRow)

                for tg in tgroups:
                    ng = len(tg)
                    ps2 = {}
                    for t in tg:
                        for half in range(2):
                            ps2[(t, half)] = psc2.tile(
                                [128, 512], f32, tag="ps2",
                                name=f"ps2_{s}_{t}_{half}")
                    r0 = base + tg[0] * 128
                    # t-major, k-chain innermost: each accumulator finishes
                    # early and staggered so its residual-add (which frees
                    # the PSUM bank) overlaps the remaining matmuls
                    for t in tg:
                        for half in range(2):
                            mm2_chain(ps2[(t, half)], t, half * 512, 512,
                                      f"{s}_{t}_{half}")
                    # z = ps2/S2 + x (residual), f16 out; LN runs on host
                    if s == NCH - 1 and tg is tgroups[-1]:
                        # latency-critical tail: write each half as soon as
                        # its residual add lands, on the cheap Pool queue
                        zh = prs.tile([128, 2 * H], f16, tag="zh",
                                      name=f"zh_{s}_{tg[0]}")
                        for half in range(2):
                            for ti, t in enumerate(tg):
                                nc.vector.scalar_tensor_tensor(
                                    out=zh[:, ti * H + half * 512:
                                           ti * H + (half + 1) * 512],
                                    in0=ps2[(t, half)][:], scalar=1.0 / S2,
                                    in1=xg_t[:, t * H + half * 512:
                                             t * H + (half + 1) * 512],
                                    op0=OP.mult, op1=OP.add)
                            nc.gpsimd.dma_start(
                                out=bass.AP(yc0.tensor,
                                            yc0.offset + r0 * H + half * 512,
                                            [[H, 128], [128 * H, ng], [1, 512]]),
                                in_=zh[:, 0:ng * H].rearrange(
                                    "p (t h) -> p t h", h=H)[
                                    :, :, half * 512:(half + 1) * 512])
                    else:
                        zh = prs.tile([128, 2 * H], f16, tag="zh",
                                      name=f"zh_{s}_{tg[0]}")
                        for ti, t in enumerate(tg):
                            for half in range(2):
                                nc.vector.scalar_tensor_tensor(
                                    out=zh[:, ti * H + half * 512:
                                           ti * H + (half + 1) * 512],
                                    in0=ps2[(t, half)][:], scalar=1.0 / S2,
                                    in1=xg_t[:, t * H + half * 512:
                                             t * H + (half + 1) * 512],
                                    op0=OP.mult, op1=OP.add)
                        nc.sync.dma_start(
                            out=bass.AP(yc0.tensor, yc0.offset + r0 * H,
                                        [[H, 128], [128 * H, ng], [1, H]]),
                            in_=zh[:, 0:ng * H].rearrange("p (t h) -> p t h", h=H))

            # ---- software pipeline: one-chunk skew keeps PE fed ----
            with tc.tile_pool(name="ps2", bufs=4, space="PSUM") as psc2:
                ring = [(psc1, "ps1")]
                for s in range(NCH):
                    if s + 2 < NCH:
                        stage_xt(s + 2, nc.sync)
                        stage_xg(s + 2, nc.gpsimd)
                    stage_mm1(s, ring)
                    if s >= 1:
                        stage_mm2_z(s - 1, psc2)
                stage_mm2_z(NCH - 1, psc2)

    if split:
        split_sync_waits(nc)
    return nc


# ---------------------------------------------------------------------------
# Host side
# ---------------------------------------------------------------------------
def plan(inputs):
    """Exact fp32 gating (replicates the reference) -> per-expert routing."""
    x = np.ascontiguousarray(
        np.asarray(inputs["x"], dtype=np.float32).reshape(-1, H))
    T = x.shape[0]
    gn_g = np.asarray(inputs["gn_g"], dtype=np.float32)
    gn_b = np.asarray(inputs["gn_b"], dtype=np.float32)
    gate_w = np.asarray(inputs["gate_w"], dtype=np.float32)
    gate_b = np.asarray(inputs["gate_b"], dtype=np.float32)

    m = x.mean(axis=1, keepdims=True, dtype=np.float32)
    d = x - m
    v = np.mean(d * d, axis=1, keepdims=True, dtype=np.float32)
    gi = d * (1.0 / np.sqrt(v + LN_EPS)) * gn_g + gn_b
    logits = gi @ gate_w + gate_b
    mx = logits.max(axis=1, keepdims=True)
    ex = np.exp(logits - mx)
    probs = ex / ex.sum(axis=1, keepdims=True)

    ar = np.arange(T)
    i1 = np.argmax(probs, axis=1)          # ties -> lower index, like top_k
    p1 = probs[ar, i1]
    pr2 = probs.copy()
    pr2[ar, i1] = -1.0
    i2 = np.argmax(pr2, axis=1)
    p2 = probs[ar, i2]
    ssum = p1 + p2 + 1e-9
    w1_, w2_ = p1 / ssum, p2 / ssum

    idx = np.concatenate([i1, i2])
    wts = np.concatenate([w1_, w2_])
    toks = np.concatenate([ar, ar])
    ids_list, wts_list = [], []
    for e in range(E):
        sel = idx == e
        te = toks[sel]
        we = wts[sel]
        order = np.argsort(te, kind="stable")
        ids_list.append(te[order].astype(np.int64))
        wts_list.append(we[order].astype(np.float32))
    counts = np.array([len(i) for i in ids_list])
    C = max(int(-(-counts.max() // 128) * 128), 128)
    return {
        "x": x,
        "ids": ids_list,
        "wts": wts_list,
        "counts": counts,
        "C": C,
        "use_b1": bool(np.any(np.asarray(inputs["b1"]))),
    }


def make_in_maps(inputs, pl):
    x = pl["x"]
    C = pl["C"]
    W1 = np.asarray(inputs["W1"], dtype=np.float32)
    b1 = np.asarray(inputs["b1"], dtype=np.float32)
    W2 = np.asarray(inputs["W2"], dtype=np.float32)

    in_maps = []
    for e in range(E):
        ids = pl["ids"][e]
        cnt = len(ids)
        xr = np.zeros((C, H), np.float32)
        xr[:cnt] = x[ids]

        m = {}
        # mm1 rhs layout: [p, j2*2C + q*C + t]
        x8 = xr.astype(ml_dtypes.float8_e4m3)
        m["xeT8"] = np.ascontiguousarray(
            x8.reshape(C, 4, 2, 128).transpose(3, 1, 2, 0).reshape(128, 8 * C))
        m["xg"] = xr.astype(np.float16)
        w1s = (W1[e] * S1).astype(ml_dtypes.float8_e4m3)
        m["w1p"] = np.ascontiguousarray(
            w1s.reshape(4, 2, 128, F).transpose(0, 2, 1, 3).reshape(4 * 128, 2 * F))
        w2s = (W2[e] * S2).astype(ml_dtypes.float8_e4m3)
        m["w2p"] = np.ascontiguousarray(
            w2s.reshape(16, 2, 128, H).transpose(0, 2, 1, 3).reshape(16 * 128, 2 * H))
        if pl["use_b1"]:
            m["b1t"] = np.ascontiguousarray(b1[e].reshape(F // 128, 128).T)
        in_maps.append(m)
    return in_maps


def combine(results, inputs, pl):
    """Host: b2 add, output LayerNorm, gate weighting, ln affine, scatter-add."""
    ln_g = np.asarray(inputs["ln_g"], dtype=np.float32)
    ln_b = np.asarray(inputs["ln_b"], dtype=np.float32)
    b2 = np.asarray(inputs["b2"], dtype=np.float32)
    T = pl["x"].shape[0]
    y = np.zeros((T, H), np.float32)
    for e, r in enumerate(results):
        cnt = int(pl["counts"][e])
        ids = pl["ids"][e][:cnt]
        w = pl["wts"][e][:cnt].astype(np.float32)
        z = np.asarray(r["Yc"][:cnt], dtype=np.float32)
        if b2[e].any():
            z += b2[e][None, :]
        m = z.mean(axis=1, keepdims=True, dtype=np.float32)
        d = z - m
        v = np.mean(d * d, axis=1, keepdims=True, dtype=np.float32)
        zn = d * (1.0 / np.sqrt(v + LN_EPS))
        y[ids] += (zn * w[:, None]) * ln_g[e][None, :] + w[:, None] * ln_b[e][None, :]
    return y


def kernel(**inputs) -> np.ndarray:
    pl = plan(inputs)
    nc = build_nc(pl["C"], use_b1=pl["use_b1"])
    in_maps = make_in_maps(inputs, pl)
    res = run_bass_kernel_spmd(nc, in_maps, core_ids=list(range(8)))
    y = combine(res.results, inputs, pl)
    return y.reshape(B, S, H)
